# revision 10
# baseline (speedup 1.0000x reference)
"""Centroid triplet loss on 8 Trainium2 NeuronCores (Bass/Tile).

Data-parallel over the batch: each of the 8 cores gets 8192 of the 65536
samples.  Per-class embedding sums and counts are all-reduced to form global
centroids; each core then computes its local triplet terms and a final
all-reduce produces the scalar loss.

Math restructure (equivalent to the reference):
    term_i = relu(margin + e_hat_i . (cent[nearest[l_i]] - cent[l_i]))
    loss   = sum_i w_{l_i} * term_i / n_present,   w_c = 1/max(count_c, 1)
Since relu(w*x) = w*relu(x) for w > 0, a sample's weighted term is
    relu(b_{l_i} + r_i * (e_i . u_{l_i}))
with u_c = w_c*(cent_near_c - cent_c), b_c = w_c*margin, r_i = 1/||e_i||.
So embeddings stay raw in SBUF; the one-hot used for the class-sum matmul is
scaled by r_i, and pass 2 gathers (u_c, b_c) rows per sample by label and
fuses the dot product via tensor_tensor_reduce.
"""

import sys

for _p in ("/opt/trn_rl_repo",):
    if _p not in sys.path:
        sys.path.insert(0, _p)

from contextlib import ExitStack

import numpy as np

from concourse import bacc, bass, mybir, tile
from concourse.bass_utils import run_bass_kernel_spmd
from concourse.masks import make_identity

F32 = mybir.dt.float32
I32 = mybir.dt.int32
ALU = mybir.AluOpType
ACTF = mybir.ActivationFunctionType

N_CORES = 8
B_FULL = 65536
D = 512
C = 256
MARGIN = 0.3
EPS = 1e-12

P = 128                      # SBUF partitions
B_LOC = B_FULL // N_CORES    # 8192 samples per core
T = B_LOC // P               # 64 sample tiles of 128
LOAD_CHUNK = 8               # tiles per embedding-load DMA (2 MiB each)
TBL_W = 520                  # table row: k*u[0:512], b, k^2*|u|^2, pad to 520
NEG = -1e30
KAPPA = 256.0                # scale for the difference-of-squares dot trick


def _build():
    nc = bacc.Bacc(
        "TRN2",
        target_bir_lowering=False,
        debug=False,
        enable_asserts=False,
        num_devices=N_CORES,
    )

    emb = nc.dram_tensor("emb", [B_LOC, D], F32, kind="ExternalInput")
    lab = nc.dram_tensor("lab", [P, T], I32, kind="ExternalInput")
    loss_out = nc.dram_tensor("loss", [1, 1], F32, kind="ExternalOutput")

    # Internal HBM scratch.  AR1 buffer: rows 0:256 per-class sums, row 256
    # carries the per-class counts in its first 256 columns.
    ar1_in = nc.dram_tensor("ar1_in", [C + 1, D], F32)
    ar1_out = nc.dram_tensor("ar1_out", [C + 1, D], F32, addr_space="Shared")
    table = nc.dram_tensor("tbl", [C, TBL_W], F32)
    ar2_in = nc.dram_tensor("ar2_in", [1, 8], F32)
    ar2_out = nc.dram_tensor("ar2_out", [1, 8], F32, addr_space="Shared")

    groups = [list(range(N_CORES))]

    with tile.TileContext(nc) as tc, ExitStack() as ctx:
        const = ctx.enter_context(tc.tile_pool(name="const", bufs=1))
        big = ctx.enter_context(tc.tile_pool(name="big", bufs=1))
        work = ctx.enter_context(tc.tile_pool(name="work", bufs=3))
        sq = ctx.enter_context(tc.tile_pool(name="sq", bufs=2))
        gat = ctx.enter_context(tc.tile_pool(name="gat", bufs=4))
        mid = ctx.enter_context(tc.tile_pool(name="mid", bufs=1))
        psacc = ctx.enter_context(tc.tile_pool(name="psacc", bufs=1, space="PSUM"))
        psmid = ctx.enter_context(tc.tile_pool(name="psmid", bufs=3, space="PSUM"))

        # ---- constants -------------------------------------------------
        ident = const.tile([P, P], F32)
        make_identity(nc, ident[:])
        iota_row = const.tile([P, C], F32)
        nc.gpsimd.iota(
            iota_row[:], pattern=[[1, C]], base=0, channel_multiplier=0,
            allow_small_or_imprecise_dtypes=True,
        )
        ones_col = const.tile([P, 1], F32)
        nc.gpsimd.memset(ones_col[:], 1.0)
        ones_row = const.tile([1, P], F32)
        nc.gpsimd.memset(ones_row[:], 1.0)

        lab_sb = const.tile([P, T], I32)
        nc.sync.dma_start(out=lab_sb[:], in_=lab.ap())
        lab_f = const.tile([P, T], F32)
        nc.vector.tensor_copy(out=lab_f[:], in_=lab_sb[:])

        # ---- pass 1: load embeddings, norms, class sums/counts ---------
        e_chunks = []
        emb_v = emb.ap().rearrange("(t p) d -> p t d", p=P)
        for ci in range(T // LOAD_CHUNK):
            ec = big.tile([P, LOAD_CHUNK, D], F32, tag=f"e{ci}")
            e_chunks.append(ec)
            sl = slice(ci * LOAD_CHUNK, (ci + 1) * LOAD_CHUNK)
            nc.sync.dma_start(out=ec[:], in_=emb_v[:, sl, :])

        norm2 = const.tile([P, T], F32)
        norm = const.tile([P, T], F32)
        r_all = const.tile([P, T], F32)

        sums_ps0 = psacc.tile([P, D], F32, tag="sums0")
        sums_ps1 = psacc.tile([P, D], F32, tag="sums1")
        cnt_ps = psacc.tile([1, C], F32, tag="cnt")

        def e_tile(t):
            return e_chunks[t // LOAD_CHUNK][:, t % LOAD_CHUNK, :]

        for t in range(T):
            et = e_tile(t)
            sq_t = sq.tile([P, D], F32, tag="sq")
            # tensor_tensor_reduce is broken on this runtime (kills the
            # exec unit) — use ACT Square with free-dim accumulation.
            nc.scalar.activation(
                sq_t[:], et, ACTF.Square, accum_out=norm2[:, t : t + 1]
            )
            nc.scalar.activation(norm[:, t : t + 1], norm2[:, t : t + 1], ACTF.Sqrt)
            nc.vector.reciprocal(r_all[:, t : t + 1], norm[:, t : t + 1])

            # r-scaled one-hot (for normalized class sums) and plain one-hot
            # (for counts)
            osc = work.tile([P, C], F32, tag="osc")
            nc.any.tensor_scalar(
                out=osc[:], in0=iota_row[:],
                scalar1=lab_f[:, t : t + 1], scalar2=r_all[:, t : t + 1],
                op0=ALU.is_equal, op1=ALU.mult,
            )
            ocnt = work.tile([P, C], F32, tag="ocnt")
            nc.gpsimd.tensor_scalar(
                out=ocnt[:], in0=iota_row[:],
                scalar1=lab_f[:, t : t + 1], scalar2=None,
                op0=ALU.is_equal,
            )
            first, last = t == 0, t == T - 1
            nc.tensor.matmul(sums_ps0[:], osc[:, 0:P], et, start=first, stop=last)
            nc.tensor.matmul(sums_ps1[:], osc[:, P:C], et, start=first, stop=last)
            nc.tensor.matmul(cnt_ps[:], ones_col[:], ocnt[:], start=first, stop=last)

        # ---- all-reduce sums + counts ----------------------------------
        sums_sb = [mid.tile([P, D], F32, tag=f"ssb{h}", name=f"ssb{h}") for h in range(2)]
        nc.vector.tensor_copy(out=sums_sb[0][:], in_=sums_ps0[:])
        nc.vector.tensor_copy(out=sums_sb[1][:], in_=sums_ps1[:])
        cnt_row = mid.tile([1, D], F32, tag="cntrow")
        nc.vector.memset(cnt_row[:], 0.0)
        nc.vector.tensor_copy(out=cnt_row[:, 0:C], in_=cnt_ps[:])

        nc.sync.dma_start(out=ar1_in.ap()[0:P, :], in_=sums_sb[0][:])
        nc.sync.dma_start(out=ar1_in.ap()[P:C, :], in_=sums_sb[1][:])
        nc.sync.dma_start(out=ar1_in.ap()[C : C + 1, :], in_=cnt_row[:])

        nc.gpsimd.collective_compute(
            "AllReduce", ALU.add, replica_groups=groups,
            ins=[ar1_in.ap()], outs=[ar1_out.ap()],
        )

        # global sums overwrite the local-sum tiles (same slots, AR is done)
        gsums = [mid.tile([P, D], F32, tag=f"ssb{h}", name=f"gs{h}") for h in range(2)]
        nc.sync.dma_start(out=gsums[0][:], in_=ar1_out.ap()[0:P, :])
        nc.sync.dma_start(out=gsums[1][:], in_=ar1_out.ap()[P:C, :])
        gcnt_row = mid.tile([1, C], F32, tag="cntrow")
        nc.sync.dma_start(out=gcnt_row[:], in_=ar1_out.ap()[C : C + 1, 0:C])

        # ---- centroids: cent = sums / max(||sums||, eps) ---------------
        cent = []
        for h in range(2):
            s2 = sq.tile([P, D], F32, tag="sq")  # scratch for the squares
            cn2 = mid.tile([P, 1], F32, tag=f"cn{h}")
            nc.scalar.activation(
                s2[:], gsums[h][:], ACTF.Square, accum_out=cn2[:]
            )
            nc.scalar.activation(cn2[:], cn2[:], ACTF.Sqrt)
            nc.vector.tensor_scalar(
                out=cn2[:], in0=cn2[:], scalar1=EPS, scalar2=None, op0=ALU.max
            )
            nc.vector.reciprocal(cn2[:], cn2[:])
            ch = mid.tile([P, D], F32, tag=f"cent{h}")
            nc.vector.tensor_scalar(
                out=ch[:], in0=gsums[h][:], scalar1=cn2[:], scalar2=None,
                op0=ALU.mult,
            )
            cent.append(ch)

        # ---- presence masks, counts columns, w -------------------------
        negmask_r = mid.tile([1, C], F32, tag="negm")
        nc.vector.tensor_scalar(
            out=negmask_r[:], in0=gcnt_row[:], scalar1=0.5, scalar2=float(NEG),
            op0=ALU.is_lt, op1=ALU.mult,
        )
        present_r = mid.tile([1, C], F32, tag="pres")
        nc.vector.tensor_scalar(
            out=present_r[:], in0=gcnt_row[:], scalar1=0.5, scalar2=None,
            op0=ALU.is_ge,
        )
        npres = mid.tile([1, 1], F32, tag="npres")
        nc.vector.reduce_sum(npres[:], present_r[:], axis=mybir.AxisListType.X)
        nc.vector.tensor_scalar(
            out=npres[:], in0=npres[:], scalar1=1.0, scalar2=None, op0=ALU.max
        )
        inv_np = mid.tile([1, 1], F32, tag="invnp")
        nc.vector.reciprocal(inv_np[:], npres[:])

        wcol = []
        for h in range(2):
            ccol_ps = psmid.tile([P, 1], F32, tag="m")
            nc.tensor.matmul(
                ccol_ps[:], gcnt_row[:, h * P : (h + 1) * P], ones_row[:, 0:1]
            )
            wc = mid.tile([P, 1], F32, tag=f"w{h}")
            nc.vector.tensor_scalar(
                out=wc[:], in0=ccol_ps[:], scalar1=1.0, scalar2=None, op0=ALU.max
            )
            nc.vector.reciprocal(wc[:], wc[:])
            wcol.append(wc)

        # ---- centroid similarity G = cent @ cent.T ---------------------
        centT = [mid.tile([P, C], F32, tag=f"ct{k}", name=f"ct{k}") for k in range(4)]
        for h in range(2):
            for k in range(4):
                tp = psmid.tile([P, P], F32, tag="m")
                nc.tensor.transpose(
                    tp[:], cent[h][:, k * P : (k + 1) * P], ident[:]
                )
                nc.vector.tensor_copy(
                    out=centT[k][:, h * P : (h + 1) * P], in_=tp[:]
                )

        g_sb = []
        for h in range(2):
            gp = psmid.tile([P, C], F32, tag="m")
            for k in range(4):
                nc.tensor.matmul(
                    gp[:], centT[k][:, h * P : (h + 1) * P], centT[k][:],
                    start=(k == 0), stop=(k == 3),
                )
            gs = mid.tile([P, C], F32, tag=f"g{h}")
            nc.vector.tensor_copy(out=gs[:], in_=gp[:])
            # mask the diagonal (self-similarity): keep where col - row != 0
            nc.gpsimd.affine_select(
                out=gs[:], in_=gs[:], compare_op=ALU.not_equal, fill=NEG,
                base=-h * P, pattern=[[1, C]], channel_multiplier=-1,
            )
            g_sb.append(gs)

        # add -1e30 to columns of empty classes (broadcast the row via PE)
        maskp = psmid.tile([P, C], F32, tag="m")
        nc.tensor.matmul(maskp[:], ones_row[:], negmask_r[:])
        for h in range(2):
            nc.vector.tensor_tensor(
                out=g_sb[h][:], in0=g_sb[h][:], in1=maskp[:], op=ALU.add
            )

        # ---- nearest-centroid one-hot (argmax by equality) -------------
        nst = [mid.tile([P, C], F32, tag=f"nst{k}", name=f"nst{k}") for k in range(2)]
        for h in range(2):
            mx = mid.tile([P, 1], F32, tag=f"mx{h}")
            nc.vector.reduce_max(mx[:], g_sb[h][:], axis=mybir.AxisListType.X)
            ns = mid.tile([P, C], F32, tag=f"ns{h}")
            nc.vector.tensor_scalar(
                out=ns[:], in0=g_sb[h][:], scalar1=mx[:], scalar2=None,
                op0=ALU.is_equal,
            )
            for k in range(2):
                tp = psmid.tile([P, P], F32, tag="m")
                nc.tensor.transpose(tp[:], ns[:, k * P : (k + 1) * P], ident[:])
                nc.vector.tensor_copy(
                    out=nst[k][:, h * P : (h + 1) * P], in_=tp[:]
                )

        # ---- u = w*(cent_near - cent), b = w*margin; write table -------
        for h in range(2):
            cnear = psmid.tile([P, D], F32, tag="m")
            for k in range(2):
                nc.tensor.matmul(
                    cnear[:], nst[k][:, h * P : (h + 1) * P], cent[k][:],
                    start=(k == 0), stop=(k == 1),
                )
            tbl_sb = mid.tile([P, TBL_W], F32, tag=f"tb{h}")
            nc.vector.tensor_tensor(
                out=tbl_sb[:, 0:D], in0=cnear[:], in1=cent[h][:], op=ALU.subtract
            )
            # k*u = (kappa*w) * (cnear - cent)
            nc.vector.tensor_scalar(
                out=tbl_sb[:, 0:D], in0=tbl_sb[:, 0:D], scalar1=wcol[h][:],
                scalar2=KAPPA, op0=ALU.mult, op1=ALU.mult,
            )
            # b = w * margin
            nc.vector.tensor_scalar(
                out=tbl_sb[:, D : D + 1], in0=wcol[h][:], scalar1=MARGIN,
                scalar2=None, op0=ALU.mult,
            )
            # k^2*|u|^2 (for recovering the dot from |e + k*u|^2)
            squ = sq.tile([P, D], F32, tag="sq")
            nc.scalar.activation(
                squ[:], tbl_sb[:, 0:D], ACTF.Square,
                accum_out=tbl_sb[:, D + 1 : D + 2],
            )
            nc.vector.memset(tbl_sb[:, D + 2 : TBL_W], 0.0)
            nc.sync.dma_start(out=table.ap()[h * P : (h + 1) * P, :], in_=tbl_sb[:])

        # ---- pass 2: gather (k*u, b, k^2|u|^2) by label; dot via the ----
        # difference of squares:  e.u = (|e + k*u|^2 - |e|^2 - k^2|u|^2)/2k.
        # (tensor_tensor_reduce is broken on HW; multi-index indirect
        # gathers too — one [P,1]-offset gather per 128-sample tile.)
        q_all = const.tile([P, T], F32)
        bu_all = const.tile([P, T, 2], F32)
        for t in range(T):
            g_t = gat.tile([P, TBL_W], F32, tag="g", name=f"g{t}")
            nc.gpsimd.indirect_dma_start(
                out=g_t[:], out_offset=None, in_=table.ap(),
                in_offset=bass.IndirectOffsetOnAxis(
                    ap=lab_sb[:, t : t + 1], axis=0
                ),
            )
            s_t = sq.tile([P, D], F32, tag="pr")
            nc.vector.tensor_tensor(
                out=s_t[:], in0=e_tile(t), in1=g_t[:, 0:D], op=ALU.add
            )
            sq2 = sq.tile([P, D], F32, tag="sq")
            nc.scalar.activation(
                sq2[:], s_t[:], ACTF.Square, accum_out=q_all[:, t : t + 1]
            )
            nc.vector.tensor_copy(out=bu_all[:, t, :], in_=g_t[:, D : D + 2])

        # pre = (q - |e|^2 - k^2|u|^2) * (r / 2k) + b ;  term = relu(pre)
        r2 = const.tile([P, T], F32)
        nc.vector.tensor_scalar(
            out=r2[:], in0=r_all[:], scalar1=1.0 / (2.0 * KAPPA), scalar2=None,
            op0=ALU.mult,
        )
        pre_all = const.tile([P, T], F32)
        nc.vector.tensor_tensor(
            out=pre_all[:], in0=q_all[:], in1=norm2[:], op=ALU.subtract
        )
        nc.vector.tensor_tensor(
            out=pre_all[:], in0=pre_all[:], in1=bu_all[:, :, 1], op=ALU.subtract
        )
        nc.vector.tensor_tensor(
            out=pre_all[:], in0=pre_all[:], in1=r2[:], op=ALU.mult
        )
        nc.vector.tensor_tensor(
            out=pre_all[:], in0=pre_all[:], in1=bu_all[:, :, 0], op=ALU.add
        )
        con_all = const.tile([P, T], F32)
        nc.scalar.activation(con_all[:], pre_all[:], ACTF.Relu)

        tot_col = mid.tile([P, 1], F32, tag="tot")
        nc.vector.reduce_sum(tot_col[:], con_all[:], axis=mybir.AxisListType.X)
        tot_ps = psmid.tile([1, 1], F32, tag="m")
        nc.tensor.matmul(tot_ps[:], tot_col[:], ones_col[:])
        tot_sb = mid.tile([1, 8], F32, tag="totsb")
        nc.vector.memset(tot_sb[:], 0.0)
        nc.vector.tensor_copy(out=tot_sb[:, 0:1], in_=tot_ps[:])
        nc.sync.dma_start(out=ar2_in.ap()[:], in_=tot_sb[:])
        nc.gpsimd.collective_compute(
            "AllReduce", ALU.add, replica_groups=groups,
            ins=[ar2_in.ap()], outs=[ar2_out.ap()],
        )
        gtot = mid.tile([1, 8], F32, tag="gtot")
        nc.sync.dma_start(out=gtot[:], in_=ar2_out.ap()[:])
        loss_sb = mid.tile([1, 1], F32, tag="loss")
        nc.vector.tensor_tensor(
            out=loss_sb[:], in0=gtot[:, 0:1], in1=inv_np[:], op=ALU.mult
        )
        nc.sync.dma_start(out=loss_out.ap()[:], in_=loss_sb[:])

    nc.compile()
    return nc


_NC = None


def _get_nc():
    global _NC
    if _NC is None:
        _NC = _build()
    return _NC


def kernel(embeddings: np.ndarray, labels: np.ndarray) -> np.ndarray:
    emb = np.ascontiguousarray(np.asarray(embeddings, dtype=np.float32))
    lab = np.asarray(labels).astype(np.int32)
    assert emb.shape == (B_FULL, D) and lab.shape == (B_FULL,)

    nc = _get_nc()
    in_maps = []
    for c in range(N_CORES):
        sl = slice(c * B_LOC, (c + 1) * B_LOC)
        lab_2d = np.ascontiguousarray(lab[sl].reshape(T, P).T)  # [P, T]
        in_maps.append({"emb": emb[sl], "lab": lab_2d})

    res = run_bass_kernel_spmd(nc, in_maps, core_ids=list(range(N_CORES)))
    loss = res.results[0]["loss"]
    return np.asarray(loss, dtype=np.float32).reshape(())


if __name__ == "__main__":
    rng = np.random.default_rng(0)
    e = rng.standard_normal((B_FULL, D), dtype=np.float32)
    l = rng.integers(0, C, size=(B_FULL,)).astype(np.int32)
    print(kernel(embeddings=e, labels=l))


# revision 11
# speedup vs baseline: 1.3298x; 1.3298x over previous
"""Centroid triplet loss on 8 Trainium2 NeuronCores (Bass/Tile).

Data-parallel over the batch: each of the 8 cores gets 8192 of the 65536
samples.  Per-class embedding sums and counts are all-reduced to form global
centroids; each core then computes its local triplet terms and a final
all-reduce produces the scalar loss.

Math restructure (equivalent to the reference):
    term_i = relu(margin + e_hat_i . (cent[nearest[l_i]] - cent[l_i]))
    loss   = sum_i w_{l_i} * term_i / n_present,   w_c = 1/max(count_c, 1)
Since relu(w*x) = w*relu(x) for w > 0, a sample's weighted term is
    relu(b_{l_i} + r_i * (e_i . u_{l_i}))
with u_c = w_c*(cent_near_c - cent_c), b_c = w_c*margin, r_i = 1/||e_i||.
So embeddings stay raw in SBUF; the one-hot used for the class-sum matmul is
scaled by r_i, and pass 2 gathers (u_c, b_c) rows per sample by label and
fuses the dot product via tensor_tensor_reduce.
"""

import sys

for _p in ("/opt/trn_rl_repo",):
    if _p not in sys.path:
        sys.path.insert(0, _p)

from contextlib import ExitStack

import numpy as np

from concourse import bacc, bass, mybir, tile
from concourse.bass_utils import run_bass_kernel_spmd
from concourse.masks import make_identity

F32 = mybir.dt.float32
I32 = mybir.dt.int32
ALU = mybir.AluOpType
ACTF = mybir.ActivationFunctionType

N_CORES = 8
B_FULL = 65536
D = 512
C = 256
MARGIN = 0.3
EPS = 1e-12

P = 128                      # SBUF partitions
B_LOC = B_FULL // N_CORES    # 8192 samples per core
T = B_LOC // P               # 64 sample tiles of 128
LOAD_CHUNK = 8               # tiles per embedding-load DMA (2 MiB each)
TBL_W = 520                  # table row: k*u[0:512], b, k^2*|u|^2, pad to 520
NEG = -1e30
KAPPA = 256.0                # scale for the difference-of-squares dot trick


def _build():
    nc = bacc.Bacc(
        "TRN2",
        target_bir_lowering=False,
        debug=False,
        enable_asserts=False,
        num_devices=N_CORES,
    )

    emb = nc.dram_tensor("emb", [B_LOC, D], F32, kind="ExternalInput")
    lab = nc.dram_tensor("lab", [P, T], I32, kind="ExternalInput")
    loss_out = nc.dram_tensor("loss", [1, 1], F32, kind="ExternalOutput")

    # Internal HBM scratch.  AR1 buffer: rows 0:256 per-class sums, row 256
    # carries the per-class counts in its first 256 columns.
    ar1_in = nc.dram_tensor("ar1_in", [C + 1, D], F32)
    ar1_out = nc.dram_tensor("ar1_out", [C + 1, D], F32, addr_space="Shared")
    table = nc.dram_tensor("tbl", [C, TBL_W], F32)
    ar2_in = nc.dram_tensor("ar2_in", [1, 8], F32)
    ar2_out = nc.dram_tensor("ar2_out", [1, 8], F32, addr_space="Shared")

    groups = [list(range(N_CORES))]

    with tile.TileContext(nc) as tc, ExitStack() as ctx:
        const = ctx.enter_context(tc.tile_pool(name="const", bufs=1))
        big = ctx.enter_context(tc.tile_pool(name="big", bufs=1))
        work = ctx.enter_context(tc.tile_pool(name="work", bufs=3))
        sq = ctx.enter_context(tc.tile_pool(name="sq", bufs=2))
        gat = ctx.enter_context(tc.tile_pool(name="gat", bufs=4))
        mid = ctx.enter_context(tc.tile_pool(name="mid", bufs=1))
        psacc = ctx.enter_context(tc.tile_pool(name="psacc", bufs=1, space="PSUM"))
        psmid = ctx.enter_context(tc.tile_pool(name="psmid", bufs=3, space="PSUM"))

        # ---- constants -------------------------------------------------
        ident = const.tile([P, P], F32)
        make_identity(nc, ident[:])
        iota_row = const.tile([P, C], F32)
        nc.gpsimd.iota(
            iota_row[:], pattern=[[1, C]], base=0, channel_multiplier=0,
            allow_small_or_imprecise_dtypes=True,
        )
        ones_col = const.tile([P, 1], F32)
        nc.gpsimd.memset(ones_col[:], 1.0)
        ones_row = const.tile([1, P], F32)
        nc.gpsimd.memset(ones_row[:], 1.0)

        lab_sb = const.tile([P, T], I32)
        nc.sync.dma_start(out=lab_sb[:], in_=lab.ap())
        lab_f = const.tile([P, T], F32)
        nc.vector.tensor_copy(out=lab_f[:], in_=lab_sb[:])

        # ---- pass 1: load embeddings, norms, class sums/counts ---------
        e_chunks = []
        emb_v = emb.ap().rearrange("(t p) d -> p t d", p=P)
        for ci in range(T // LOAD_CHUNK):
            ec = big.tile([P, LOAD_CHUNK, D], F32, tag=f"e{ci}")
            e_chunks.append(ec)
            sl = slice(ci * LOAD_CHUNK, (ci + 1) * LOAD_CHUNK)
            nc.sync.dma_start(out=ec[:], in_=emb_v[:, sl, :])

        norm2 = const.tile([P, T], F32)
        norm = const.tile([P, T], F32)
        r_all = const.tile([P, T], F32)

        sums_ps0 = psacc.tile([P, D], F32, tag="sums0")
        sums_ps1 = psacc.tile([P, D], F32, tag="sums1")
        cnt_ps = psacc.tile([1, C], F32, tag="cnt")

        def e_tile(t):
            return e_chunks[t // LOAD_CHUNK][:, t % LOAD_CHUNK, :]

        for t in range(T):
            et = e_tile(t)
            sq_t = sq.tile([P, D], F32, tag="sq")
            # tensor_tensor_reduce is broken on this runtime (kills the
            # exec unit) — use ACT Square with free-dim accumulation.
            nc.scalar.activation(
                sq_t[:], et, ACTF.Square, accum_out=norm2[:, t : t + 1]
            )
            nc.scalar.activation(norm[:, t : t + 1], norm2[:, t : t + 1], ACTF.Sqrt)
            nc.vector.reciprocal(r_all[:, t : t + 1], norm[:, t : t + 1])

            # plain one-hot (tensor_scalar is ~10x slower than broadcast
            # tensor_tensor on both DVE and GpSimd — use TT)
            oht = work.tile([P, C], F32, tag="oht")
            nc.vector.tensor_tensor(
                out=oht[:], in0=iota_row[:],
                in1=lab_f[:, t : t + 1].to_broadcast([P, C]), op=ALU.is_equal,
            )
            # r-scaled one-hot for the normalized class sums; alternate the
            # scaling between ACT and DVE to balance engine load
            osc = work.tile([P, C], F32, tag="osc")
            if t % 2 == 0:
                nc.scalar.activation(
                    osc[:], oht[:], ACTF.Copy, scale=r_all[:, t : t + 1]
                )
            else:
                nc.vector.tensor_tensor(
                    out=osc[:], in0=oht[:],
                    in1=r_all[:, t : t + 1].to_broadcast([P, C]), op=ALU.mult,
                )
            first, last = t == 0, t == T - 1
            nc.tensor.matmul(sums_ps0[:], osc[:, 0:P], et, start=first, stop=last)
            nc.tensor.matmul(sums_ps1[:], osc[:, P:C], et, start=first, stop=last)
            nc.tensor.matmul(cnt_ps[:], ones_col[:], oht[:], start=first, stop=last)

        # ---- all-reduce sums + counts ----------------------------------
        sums_sb = [mid.tile([P, D], F32, tag=f"ssb{h}", name=f"ssb{h}") for h in range(2)]
        nc.vector.tensor_copy(out=sums_sb[0][:], in_=sums_ps0[:])
        nc.vector.tensor_copy(out=sums_sb[1][:], in_=sums_ps1[:])
        cnt_row = mid.tile([1, D], F32, tag="cntrow")
        nc.vector.memset(cnt_row[:], 0.0)
        nc.vector.tensor_copy(out=cnt_row[:, 0:C], in_=cnt_ps[:])

        nc.sync.dma_start(out=ar1_in.ap()[0:P, :], in_=sums_sb[0][:])
        nc.sync.dma_start(out=ar1_in.ap()[P:C, :], in_=sums_sb[1][:])
        nc.sync.dma_start(out=ar1_in.ap()[C : C + 1, :], in_=cnt_row[:])

        nc.gpsimd.collective_compute(
            "AllReduce", ALU.add, replica_groups=groups,
            ins=[ar1_in.ap()], outs=[ar1_out.ap()],
        )

        # global sums overwrite the local-sum tiles (same slots, AR is done)
        gsums = [mid.tile([P, D], F32, tag=f"ssb{h}", name=f"gs{h}") for h in range(2)]
        nc.sync.dma_start(out=gsums[0][:], in_=ar1_out.ap()[0:P, :])
        nc.sync.dma_start(out=gsums[1][:], in_=ar1_out.ap()[P:C, :])
        gcnt_row = mid.tile([1, C], F32, tag="cntrow")
        nc.sync.dma_start(out=gcnt_row[:], in_=ar1_out.ap()[C : C + 1, 0:C])

        # ---- centroids: cent = sums / max(||sums||, eps) ---------------
        cent = []
        for h in range(2):
            s2 = sq.tile([P, D], F32, tag="sq")  # scratch for the squares
            cn2 = mid.tile([P, 1], F32, tag=f"cn{h}")
            nc.scalar.activation(
                s2[:], gsums[h][:], ACTF.Square, accum_out=cn2[:]
            )
            nc.scalar.activation(cn2[:], cn2[:], ACTF.Sqrt)
            nc.vector.tensor_scalar(
                out=cn2[:], in0=cn2[:], scalar1=EPS, scalar2=None, op0=ALU.max
            )
            nc.vector.reciprocal(cn2[:], cn2[:])
            ch = mid.tile([P, D], F32, tag=f"cent{h}")
            nc.vector.tensor_scalar(
                out=ch[:], in0=gsums[h][:], scalar1=cn2[:], scalar2=None,
                op0=ALU.mult,
            )
            cent.append(ch)

        # ---- presence masks, counts columns, w -------------------------
        negmask_r = mid.tile([1, C], F32, tag="negm")
        nc.vector.tensor_scalar(
            out=negmask_r[:], in0=gcnt_row[:], scalar1=0.5, scalar2=float(NEG),
            op0=ALU.is_lt, op1=ALU.mult,
        )
        present_r = mid.tile([1, C], F32, tag="pres")
        nc.vector.tensor_scalar(
            out=present_r[:], in0=gcnt_row[:], scalar1=0.5, scalar2=None,
            op0=ALU.is_ge,
        )
        npres = mid.tile([1, 1], F32, tag="npres")
        nc.vector.reduce_sum(npres[:], present_r[:], axis=mybir.AxisListType.X)
        nc.vector.tensor_scalar(
            out=npres[:], in0=npres[:], scalar1=1.0, scalar2=None, op0=ALU.max
        )
        inv_np = mid.tile([1, 1], F32, tag="invnp")
        nc.vector.reciprocal(inv_np[:], npres[:])

        wcol = []
        for h in range(2):
            ccol_ps = psmid.tile([P, 1], F32, tag="m")
            nc.tensor.matmul(
                ccol_ps[:], gcnt_row[:, h * P : (h + 1) * P], ones_row[:, 0:1]
            )
            wc = mid.tile([P, 1], F32, tag=f"w{h}")
            nc.vector.tensor_scalar(
                out=wc[:], in0=ccol_ps[:], scalar1=1.0, scalar2=None, op0=ALU.max
            )
            nc.vector.reciprocal(wc[:], wc[:])
            wcol.append(wc)

        # ---- centroid similarity G = cent @ cent.T ---------------------
        centT = [mid.tile([P, C], F32, tag=f"ct{k}", name=f"ct{k}") for k in range(4)]
        for h in range(2):
            for k in range(4):
                tp = psmid.tile([P, P], F32, tag="m")
                nc.tensor.transpose(
                    tp[:], cent[h][:, k * P : (k + 1) * P], ident[:]
                )
                nc.vector.tensor_copy(
                    out=centT[k][:, h * P : (h + 1) * P], in_=tp[:]
                )

        g_sb = []
        for h in range(2):
            gp = psmid.tile([P, C], F32, tag="m")
            for k in range(4):
                nc.tensor.matmul(
                    gp[:], centT[k][:, h * P : (h + 1) * P], centT[k][:],
                    start=(k == 0), stop=(k == 3),
                )
            gs = mid.tile([P, C], F32, tag=f"g{h}")
            nc.vector.tensor_copy(out=gs[:], in_=gp[:])
            # mask the diagonal (self-similarity): keep where col - row != 0
            nc.gpsimd.affine_select(
                out=gs[:], in_=gs[:], compare_op=ALU.not_equal, fill=NEG,
                base=-h * P, pattern=[[1, C]], channel_multiplier=-1,
            )
            g_sb.append(gs)

        # add -1e30 to columns of empty classes (broadcast the row via PE)
        maskp = psmid.tile([P, C], F32, tag="m")
        nc.tensor.matmul(maskp[:], ones_row[:], negmask_r[:])
        for h in range(2):
            nc.vector.tensor_tensor(
                out=g_sb[h][:], in0=g_sb[h][:], in1=maskp[:], op=ALU.add
            )

        # ---- nearest-centroid one-hot (argmax by equality) -------------
        nst = [mid.tile([P, C], F32, tag=f"nst{k}", name=f"nst{k}") for k in range(2)]
        for h in range(2):
            mx = mid.tile([P, 1], F32, tag=f"mx{h}")
            nc.vector.reduce_max(mx[:], g_sb[h][:], axis=mybir.AxisListType.X)
            ns = mid.tile([P, C], F32, tag=f"ns{h}")
            nc.vector.tensor_scalar(
                out=ns[:], in0=g_sb[h][:], scalar1=mx[:], scalar2=None,
                op0=ALU.is_equal,
            )
            for k in range(2):
                tp = psmid.tile([P, P], F32, tag="m")
                nc.tensor.transpose(tp[:], ns[:, k * P : (k + 1) * P], ident[:])
                nc.vector.tensor_copy(
                    out=nst[k][:, h * P : (h + 1) * P], in_=tp[:]
                )

        # ---- u = w*(cent_near - cent), b = w*margin; write table -------
        for h in range(2):
            cnear = psmid.tile([P, D], F32, tag="m")
            for k in range(2):
                nc.tensor.matmul(
                    cnear[:], nst[k][:, h * P : (h + 1) * P], cent[k][:],
                    start=(k == 0), stop=(k == 1),
                )
            tbl_sb = mid.tile([P, TBL_W], F32, tag=f"tb{h}")
            nc.vector.tensor_tensor(
                out=tbl_sb[:, 0:D], in0=cnear[:], in1=cent[h][:], op=ALU.subtract
            )
            # k*u = (kappa*w) * (cnear - cent)
            nc.vector.tensor_scalar(
                out=tbl_sb[:, 0:D], in0=tbl_sb[:, 0:D], scalar1=wcol[h][:],
                scalar2=KAPPA, op0=ALU.mult, op1=ALU.mult,
            )
            # b = w * margin
            nc.vector.tensor_scalar(
                out=tbl_sb[:, D : D + 1], in0=wcol[h][:], scalar1=MARGIN,
                scalar2=None, op0=ALU.mult,
            )
            # k^2*|u|^2 (for recovering the dot from |e + k*u|^2)
            squ = sq.tile([P, D], F32, tag="sq")
            nc.scalar.activation(
                squ[:], tbl_sb[:, 0:D], ACTF.Square,
                accum_out=tbl_sb[:, D + 1 : D + 2],
            )
            nc.vector.memset(tbl_sb[:, D + 2 : TBL_W], 0.0)
            nc.sync.dma_start(out=table.ap()[h * P : (h + 1) * P, :], in_=tbl_sb[:])

        # ---- pass 2: gather (k*u, b, k^2|u|^2) by label; dot via the ----
        # difference of squares:  e.u = (|e + k*u|^2 - |e|^2 - k^2|u|^2)/2k.
        # (tensor_tensor_reduce is broken on HW; multi-index indirect
        # gathers too — one [P,1]-offset gather per 128-sample tile.)
        q_all = const.tile([P, T], F32)
        bu_all = const.tile([P, T, 2], F32)
        for t in range(T):
            g_t = gat.tile([P, TBL_W], F32, tag="g", name=f"g{t}")
            nc.gpsimd.indirect_dma_start(
                out=g_t[:], out_offset=None, in_=table.ap(),
                in_offset=bass.IndirectOffsetOnAxis(
                    ap=lab_sb[:, t : t + 1], axis=0
                ),
            )
            s_t = sq.tile([P, D], F32, tag="pr")
            nc.vector.tensor_tensor(
                out=s_t[:], in0=e_tile(t), in1=g_t[:, 0:D], op=ALU.add
            )
            sq2 = sq.tile([P, D], F32, tag="sq")
            nc.scalar.activation(
                sq2[:], s_t[:], ACTF.Square, accum_out=q_all[:, t : t + 1]
            )
            nc.vector.tensor_copy(out=bu_all[:, t, :], in_=g_t[:, D : D + 2])

        # pre = (q - |e|^2 - k^2|u|^2) * (r / 2k) + b ;  term = relu(pre)
        r2 = const.tile([P, T], F32)
        nc.vector.tensor_scalar(
            out=r2[:], in0=r_all[:], scalar1=1.0 / (2.0 * KAPPA), scalar2=None,
            op0=ALU.mult,
        )
        pre_all = const.tile([P, T], F32)
        nc.vector.tensor_tensor(
            out=pre_all[:], in0=q_all[:], in1=norm2[:], op=ALU.subtract
        )
        nc.vector.tensor_tensor(
            out=pre_all[:], in0=pre_all[:], in1=bu_all[:, :, 1], op=ALU.subtract
        )
        nc.vector.tensor_tensor(
            out=pre_all[:], in0=pre_all[:], in1=r2[:], op=ALU.mult
        )
        nc.vector.tensor_tensor(
            out=pre_all[:], in0=pre_all[:], in1=bu_all[:, :, 0], op=ALU.add
        )
        con_all = const.tile([P, T], F32)
        nc.scalar.activation(con_all[:], pre_all[:], ACTF.Relu)

        tot_col = mid.tile([P, 1], F32, tag="tot")
        nc.vector.reduce_sum(tot_col[:], con_all[:], axis=mybir.AxisListType.X)
        tot_ps = psmid.tile([1, 1], F32, tag="m")
        nc.tensor.matmul(tot_ps[:], tot_col[:], ones_col[:])
        tot_sb = mid.tile([1, 8], F32, tag="totsb")
        nc.vector.memset(tot_sb[:], 0.0)
        nc.vector.tensor_copy(out=tot_sb[:, 0:1], in_=tot_ps[:])
        nc.sync.dma_start(out=ar2_in.ap()[:], in_=tot_sb[:])
        nc.gpsimd.collective_compute(
            "AllReduce", ALU.add, replica_groups=groups,
            ins=[ar2_in.ap()], outs=[ar2_out.ap()],
        )
        gtot = mid.tile([1, 8], F32, tag="gtot")
        nc.sync.dma_start(out=gtot[:], in_=ar2_out.ap()[:])
        loss_sb = mid.tile([1, 1], F32, tag="loss")
        nc.vector.tensor_tensor(
            out=loss_sb[:], in0=gtot[:, 0:1], in1=inv_np[:], op=ALU.mult
        )
        nc.sync.dma_start(out=loss_out.ap()[:], in_=loss_sb[:])

    nc.compile()
    return nc


_NC = None


def _get_nc():
    global _NC
    if _NC is None:
        _NC = _build()
    return _NC


def kernel(embeddings: np.ndarray, labels: np.ndarray) -> np.ndarray:
    emb = np.ascontiguousarray(np.asarray(embeddings, dtype=np.float32))
    lab = np.asarray(labels).astype(np.int32)
    assert emb.shape == (B_FULL, D) and lab.shape == (B_FULL,)

    nc = _get_nc()
    in_maps = []
    for c in range(N_CORES):
        sl = slice(c * B_LOC, (c + 1) * B_LOC)
        lab_2d = np.ascontiguousarray(lab[sl].reshape(T, P).T)  # [P, T]
        in_maps.append({"emb": emb[sl], "lab": lab_2d})

    res = run_bass_kernel_spmd(nc, in_maps, core_ids=list(range(N_CORES)))
    loss = res.results[0]["loss"]
    return np.asarray(loss, dtype=np.float32).reshape(())


if __name__ == "__main__":
    rng = np.random.default_rng(0)
    e = rng.standard_normal((B_FULL, D), dtype=np.float32)
    l = rng.integers(0, C, size=(B_FULL,)).astype(np.int32)
    print(kernel(embeddings=e, labels=l))


# revision 17
# speedup vs baseline: 1.4851x; 1.1168x over previous
"""Centroid triplet loss on 8 Trainium2 NeuronCores (Bass/Tile).

Data-parallel over the batch: each of the 8 cores gets 8192 of the 65536
samples.  Per-class embedding sums and counts are all-reduced to form global
centroids; each core then computes its local triplet terms and a final
all-reduce produces the scalar loss.

Math restructure (equivalent to the reference):
    term_i = relu(margin + e_hat_i . (cent[nearest[l_i]] - cent[l_i]))
    loss   = sum_i w_{l_i} * term_i / n_present,   w_c = 1/max(count_c, 1)
Since relu(w*x) = w*relu(x) for w > 0, a sample's weighted term is
    relu(b_{l_i} + r_i * (e_i . u_{l_i}))
with u_c = w_c*(cent_near_c - cent_c), b_c = w_c*margin, r_i = 1/||e_i||.
So embeddings stay raw in SBUF; the one-hot used for the class-sum matmul is
scaled by r_i, and pass 2 gathers (u_c, b_c) rows per sample by label and
fuses the dot product via tensor_tensor_reduce.
"""

import sys

for _p in ("/opt/trn_rl_repo",):
    if _p not in sys.path:
        sys.path.insert(0, _p)

from contextlib import ExitStack

import numpy as np

from concourse import bacc, bass, mybir, tile
from concourse.bass_utils import run_bass_kernel_spmd
from concourse.masks import make_identity

F32 = mybir.dt.float32
BF16 = mybir.dt.bfloat16
I32 = mybir.dt.int32
I16 = mybir.dt.int16
ALU = mybir.AluOpType
ACTF = mybir.ActivationFunctionType

N_CORES = 8
B_FULL = 65536
D = 512
C = 256
MARGIN = 0.3
EPS = 1e-12

P = 128                      # SBUF partitions
B_LOC = B_FULL // N_CORES    # 8192 samples per core
T = B_LOC // P               # 64 sample tiles of 128
LOAD_CHUNK = 8               # tiles per embedding-load DMA (2 MiB each)
TBL_W = 576                  # table row: k*u[0:512], b, k^2*|u|^2, pad to 576 (2304B, mult of 256B for dma_gather)
GCHUNK = 1024                # indices per dma_gather call (8 sample tiles)
NEG = -1e30
KAPPA = 256.0                # scale for the difference-of-squares dot trick


def _build():
    nc = bacc.Bacc(
        "TRN2",
        target_bir_lowering=False,
        debug=False,
        enable_asserts=False,
        num_devices=N_CORES,
    )

    emb = nc.dram_tensor("emb", [B_LOC, D], F32, kind="ExternalInput")
    lab = nc.dram_tensor("lab", [P, T], I32, kind="ExternalInput")
    # labels in dma_gather's wrapped-int16 layout: idx i lives at
    # [i % 16, i // 16], replicated into all eight 16-partition groups
    lab16 = nc.dram_tensor("lab16", [P, B_LOC // 16], I16, kind="ExternalInput")
    loss_out = nc.dram_tensor("loss", [1, 1], F32, kind="ExternalOutput")

    # Internal HBM scratch.  AR1 buffer: rows 0:256 per-class sums, row 256
    # carries the per-class counts in its first 256 columns.
    ar1_in = nc.dram_tensor("ar1_in", [C + 1, D], F32)
    ar1_out = nc.dram_tensor("ar1_out", [C + 1, D], F32, addr_space="Shared")
    table = nc.dram_tensor("tbl", [C, TBL_W], F32)
    ar2_in = nc.dram_tensor("ar2_in", [1, 8], F32)
    ar2_out = nc.dram_tensor("ar2_out", [1, 8], F32, addr_space="Shared")

    groups = [list(range(N_CORES))]

    with tile.TileContext(nc) as tc, ExitStack() as ctx:
        const = ctx.enter_context(tc.tile_pool(name="const", bufs=1))
        big = ctx.enter_context(tc.tile_pool(name="big", bufs=1))
        work = ctx.enter_context(tc.tile_pool(name="work", bufs=3))
        sq = ctx.enter_context(tc.tile_pool(name="sq", bufs=2))
        gat = ctx.enter_context(tc.tile_pool(name="gat", bufs=4))
        mid = ctx.enter_context(tc.tile_pool(name="mid", bufs=1))
        psacc = ctx.enter_context(tc.tile_pool(name="psacc", bufs=1, space="PSUM"))
        psmid = ctx.enter_context(tc.tile_pool(name="psmid", bufs=3, space="PSUM"))

        # ---- constants -------------------------------------------------
        ident = const.tile([P, P], F32)
        make_identity(nc, ident[:])
        iota_row = const.tile([P, C], BF16)
        nc.gpsimd.iota(
            iota_row[:], pattern=[[1, C]], base=0, channel_multiplier=0,
            allow_small_or_imprecise_dtypes=True,
        )
        ones_col = const.tile([P, 1], F32)
        nc.gpsimd.memset(ones_col[:], 1.0)
        ones_col_bf = const.tile([P, 1], BF16)
        nc.gpsimd.memset(ones_col_bf[:], 1.0)
        ones_row = const.tile([1, P], F32)
        nc.gpsimd.memset(ones_row[:], 1.0)

        lab_sb = const.tile([P, T], I32)
        nc.sync.dma_start(out=lab_sb[:], in_=lab.ap())
        lab_f = const.tile([P, T], BF16)
        nc.vector.tensor_copy(out=lab_f[:], in_=lab_sb[:])
        lab16_sb = const.tile([P, B_LOC // 16], I16)
        nc.sync.dma_start(out=lab16_sb[:], in_=lab16.ap())

        # ---- pass 1: load embeddings, norms, class sums/counts ---------
        e_chunks = []
        emb_v = emb.ap().rearrange("(t p) d -> p t d", p=P)
        for ci in range(T // LOAD_CHUNK):
            # bf16 residency: halves SBUF and lets the class-sum matmuls run
            # single-pass bf16 instead of fp32 HI/LO pairs (cast in the DMA,
            # SWDGE-only feature)
            ec = big.tile([P, LOAD_CHUNK, D], BF16, tag=f"e{ci}")
            e_chunks.append(ec)
            sl = slice(ci * LOAD_CHUNK, (ci + 1) * LOAD_CHUNK)
            nc.gpsimd.dma_start(out=ec[:], in_=emb_v[:, sl, :])

        norm2 = const.tile([P, T], F32)
        norm = const.tile([P, T], F32)
        r_all = const.tile([P, T], F32)
        r_bf = const.tile([P, T], BF16)

        sums_ps0 = psacc.tile([P, D], F32, tag="sums0")
        sums_ps1 = psacc.tile([P, D], F32, tag="sums1")
        cnt_ps = psacc.tile([1, C], F32, tag="cnt")

        def e_tile(t):
            return e_chunks[t // LOAD_CHUNK][:, t % LOAD_CHUNK, :]

        for ci in range(T // LOAD_CHUNK):
            csl = slice(ci * LOAD_CHUNK, (ci + 1) * LOAD_CHUNK)
            for j in range(LOAD_CHUNK):
                t = ci * LOAD_CHUNK + j
                sq_t = sq.tile([P, D], F32, tag="sq")
                # tensor_tensor_reduce is broken on this runtime (kills the
                # exec unit) — use ACT Square with free-dim accumulation.
                nc.scalar.activation(
                    sq_t[:], e_tile(t), ACTF.Square,
                    accum_out=norm2[:, t : t + 1],
                )
            # batched per-chunk norm -> r (cheaper than per-tile column ops)
            nc.scalar.activation(norm[:, csl], norm2[:, csl], ACTF.Sqrt)
            nc.vector.reciprocal(r_all[:, csl], norm[:, csl])
            nc.vector.tensor_copy(out=r_bf[:, csl], in_=r_all[:, csl])

            for j in range(LOAD_CHUNK):
                t = ci * LOAD_CHUNK + j
                et = e_tile(t)
                # plain one-hot (tensor_scalar is ~10x slower than broadcast
                # tensor_tensor — use TT against a bf16 iota)
                oht = work.tile([P, C], BF16, tag="oht")
                nc.vector.tensor_tensor(
                    out=oht[:], in0=iota_row[:],
                    in1=lab_f[:, t : t + 1].to_broadcast([P, C]),
                    op=ALU.is_equal,
                )
                # r-scaled one-hot for the normalized class sums; alternate
                # the scaling between ACT and DVE to balance engine load
                osc = work.tile([P, C], BF16, tag="osc")
                if t % 2 == 0:
                    nc.scalar.activation(
                        osc[:], oht[:], ACTF.Copy, scale=r_all[:, t : t + 1]
                    )
                else:
                    nc.vector.tensor_tensor(
                        out=osc[:], in0=oht[:],
                        in1=r_bf[:, t : t + 1].to_broadcast([P, C]),
                        op=ALU.mult,
                    )
                first, last = t == 0, t == T - 1
                nc.tensor.matmul(
                    sums_ps0[:], osc[:, 0:P], et, start=first, stop=last
                )
                nc.tensor.matmul(
                    sums_ps1[:], osc[:, P:C], et, start=first, stop=last
                )
                nc.tensor.matmul(
                    cnt_ps[:], ones_col_bf[:], oht[:], start=first, stop=last
                )

        # ---- all-reduce sums + counts ----------------------------------
        sums_sb = [mid.tile([P, D], F32, tag=f"ssb{h}", name=f"ssb{h}") for h in range(2)]
        nc.vector.tensor_copy(out=sums_sb[0][:], in_=sums_ps0[:])
        nc.vector.tensor_copy(out=sums_sb[1][:], in_=sums_ps1[:])
        cnt_row = mid.tile([1, D], F32, tag="cntrow")
        nc.vector.memset(cnt_row[:], 0.0)
        nc.vector.tensor_copy(out=cnt_row[:, 0:C], in_=cnt_ps[:])

        nc.sync.dma_start(out=ar1_in.ap()[0:P, :], in_=sums_sb[0][:])
        nc.sync.dma_start(out=ar1_in.ap()[P:C, :], in_=sums_sb[1][:])
        nc.sync.dma_start(out=ar1_in.ap()[C : C + 1, :], in_=cnt_row[:])

        nc.gpsimd.collective_compute(
            "AllReduce", ALU.add, replica_groups=groups,
            ins=[ar1_in.ap()], outs=[ar1_out.ap()],
        )

        # global sums overwrite the local-sum tiles (same slots, AR is done)
        gsums = [mid.tile([P, D], F32, tag=f"ssb{h}", name=f"gs{h}") for h in range(2)]
        nc.sync.dma_start(out=gsums[0][:], in_=ar1_out.ap()[0:P, :])
        nc.sync.dma_start(out=gsums[1][:], in_=ar1_out.ap()[P:C, :])
        gcnt_row = mid.tile([1, C], F32, tag="cntrow")
        nc.sync.dma_start(out=gcnt_row[:], in_=ar1_out.ap()[C : C + 1, 0:C])

        # ---- centroids: cent = sums / max(||sums||, eps) ---------------
        cent = []
        for h in range(2):
            s2 = sq.tile([P, D], F32, tag="sq")  # scratch for the squares
            cn2 = mid.tile([P, 1], F32, tag=f"cn{h}")
            nc.scalar.activation(
                s2[:], gsums[h][:], ACTF.Square, accum_out=cn2[:]
            )
            nc.scalar.activation(cn2[:], cn2[:], ACTF.Sqrt)
            nc.vector.tensor_scalar(
                out=cn2[:], in0=cn2[:], scalar1=EPS, scalar2=None, op0=ALU.max
            )
            nc.vector.reciprocal(cn2[:], cn2[:])
            ch = mid.tile([P, D], F32, tag=f"cent{h}")
            nc.vector.tensor_scalar(
                out=ch[:], in0=gsums[h][:], scalar1=cn2[:], scalar2=None,
                op0=ALU.mult,
            )
            cent.append(ch)

        # ---- presence masks, counts columns, w -------------------------
        negmask_r = mid.tile([1, C], F32, tag="negm")
        nc.vector.tensor_scalar(
            out=negmask_r[:], in0=gcnt_row[:], scalar1=0.5, scalar2=float(NEG),
            op0=ALU.is_lt, op1=ALU.mult,
        )
        present_r = mid.tile([1, C], F32, tag="pres")
        nc.vector.tensor_scalar(
            out=present_r[:], in0=gcnt_row[:], scalar1=0.5, scalar2=None,
            op0=ALU.is_ge,
        )
        npres = mid.tile([1, 1], F32, tag="npres")
        nc.vector.reduce_sum(npres[:], present_r[:], axis=mybir.AxisListType.X)
        nc.vector.tensor_scalar(
            out=npres[:], in0=npres[:], scalar1=1.0, scalar2=None, op0=ALU.max
        )
        inv_np = mid.tile([1, 1], F32, tag="invnp")
        nc.vector.reciprocal(inv_np[:], npres[:])

        wcol = []
        for h in range(2):
            ccol_ps = psmid.tile([P, 1], F32, tag="m")
            nc.tensor.matmul(
                ccol_ps[:], gcnt_row[:, h * P : (h + 1) * P], ones_row[:, 0:1]
            )
            wc = mid.tile([P, 1], F32, tag=f"w{h}")
            nc.vector.tensor_scalar(
                out=wc[:], in0=ccol_ps[:], scalar1=1.0, scalar2=None, op0=ALU.max
            )
            nc.vector.reciprocal(wc[:], wc[:])
            wcol.append(wc)

        # ---- centroid similarity G = cent @ cent.T ---------------------
        centT = [mid.tile([P, C], F32, tag=f"ct{k}", name=f"ct{k}") for k in range(4)]
        for h in range(2):
            for k in range(4):
                tp = psmid.tile([P, P], F32, tag="m")
                nc.tensor.transpose(
                    tp[:], cent[h][:, k * P : (k + 1) * P], ident[:]
                )
                nc.vector.tensor_copy(
                    out=centT[k][:, h * P : (h + 1) * P], in_=tp[:]
                )

        g_sb = []
        for h in range(2):
            gp = psmid.tile([P, C], F32, tag="m")
            for k in range(4):
                nc.tensor.matmul(
                    gp[:], centT[k][:, h * P : (h + 1) * P], centT[k][:],
                    start=(k == 0), stop=(k == 3),
                )
            gs = mid.tile([P, C], F32, tag=f"g{h}")
            nc.vector.tensor_copy(out=gs[:], in_=gp[:])
            # mask the diagonal (self-similarity): keep where col - row != 0
            nc.gpsimd.affine_select(
                out=gs[:], in_=gs[:], compare_op=ALU.not_equal, fill=NEG,
                base=-h * P, pattern=[[1, C]], channel_multiplier=-1,
            )
            g_sb.append(gs)

        # add -1e30 to columns of empty classes (broadcast the row via PE)
        maskp = psmid.tile([P, C], F32, tag="m")
        nc.tensor.matmul(maskp[:], ones_row[:], negmask_r[:])
        for h in range(2):
            nc.vector.tensor_tensor(
                out=g_sb[h][:], in0=g_sb[h][:], in1=maskp[:], op=ALU.add
            )

        # ---- nearest-centroid one-hot (argmax by equality) -------------
        nst = [mid.tile([P, C], F32, tag=f"nst{k}", name=f"nst{k}") for k in range(2)]
        for h in range(2):
            mx = mid.tile([P, 1], F32, tag=f"mx{h}")
            nc.vector.reduce_max(mx[:], g_sb[h][:], axis=mybir.AxisListType.X)
            ns = mid.tile([P, C], F32, tag=f"ns{h}")
            nc.vector.tensor_scalar(
                out=ns[:], in0=g_sb[h][:], scalar1=mx[:], scalar2=None,
                op0=ALU.is_equal,
            )
            for k in range(2):
                tp = psmid.tile([P, P], F32, tag="m")
                nc.tensor.transpose(tp[:], ns[:, k * P : (k + 1) * P], ident[:])
                nc.vector.tensor_copy(
                    out=nst[k][:, h * P : (h + 1) * P], in_=tp[:]
                )

        # ---- u = w*(cent_near - cent), b = w*margin; write table -------
        for h in range(2):
            cnear = psmid.tile([P, D], F32, tag="m")
            for k in range(2):
                nc.tensor.matmul(
                    cnear[:], nst[k][:, h * P : (h + 1) * P], cent[k][:],
                    start=(k == 0), stop=(k == 1),
                )
            tbl_sb = mid.tile([P, TBL_W], F32, tag=f"tb{h}")
            nc.vector.tensor_tensor(
                out=tbl_sb[:, 0:D], in0=cnear[:], in1=cent[h][:], op=ALU.subtract
            )
            # k*u = (kappa*w) * (cnear - cent)
            nc.vector.tensor_scalar(
                out=tbl_sb[:, 0:D], in0=tbl_sb[:, 0:D], scalar1=wcol[h][:],
                scalar2=KAPPA, op0=ALU.mult, op1=ALU.mult,
            )
            # b = w * margin
            nc.vector.tensor_scalar(
                out=tbl_sb[:, D : D + 1], in0=wcol[h][:], scalar1=MARGIN,
                scalar2=None, op0=ALU.mult,
            )
            # k^2*|u|^2 (for recovering the dot from |e + k*u|^2)
            squ = sq.tile([P, D], F32, tag="sq")
            nc.scalar.activation(
                squ[:], tbl_sb[:, 0:D], ACTF.Square,
                accum_out=tbl_sb[:, D + 1 : D + 2],
            )
            nc.vector.memset(tbl_sb[:, D + 2 : TBL_W], 0.0)
            nc.sync.dma_start(out=table.ap()[h * P : (h + 1) * P, :], in_=tbl_sb[:])

        # ---- pass 2: gather (k*u, b, k^2|u|^2) by label; dot via the ----
        # difference of squares:  e.u = (|e + k*u|^2 - |e|^2 - k^2|u|^2)/2k.
        # (tensor_tensor_reduce is broken on HW; multi-index indirect
        # gathers too — one [P,1]-offset gather per 128-sample tile.)
        q_all = const.tile([P, T], F32)
        bu_all = const.tile([P, T, 2], F32)
        tiles_per_g = GCHUNK // P
        for gc in range(T // tiles_per_g):
            g_t = gat.tile([P, tiles_per_g, TBL_W], F32, tag="g", name=f"g{gc}")
            nc.gpsimd.dma_gather(
                out_ap=g_t[:], in_ap=table.ap(),
                idxs_ap=lab16_sb[:, gc * (GCHUNK // 16) : (gc + 1) * (GCHUNK // 16)],
                num_idxs=GCHUNK, num_idxs_reg=GCHUNK, elem_size=TBL_W,
                single_packet=False,
            )
            nc.vector.tensor_copy(
                out=bu_all[:, gc * tiles_per_g : (gc + 1) * tiles_per_g, :],
                in_=g_t[:, :, D : D + 2],
            )
            for j in range(tiles_per_g):
                t = gc * tiles_per_g + j
                s_t = sq.tile([P, D], F32, tag="pr")
                nc.vector.tensor_tensor(
                    out=s_t[:], in0=e_tile(t), in1=g_t[:, j, 0:D], op=ALU.add
                )
                sq2 = sq.tile([P, D], F32, tag="sq")
                nc.scalar.activation(
                    sq2[:], s_t[:], ACTF.Square, accum_out=q_all[:, t : t + 1]
                )


        # pre = (q - |e|^2 - k^2|u|^2) * (r / 2k) + b ;  term = relu(pre)
        r2 = const.tile([P, T], F32)
        nc.vector.tensor_scalar(
            out=r2[:], in0=r_all[:], scalar1=1.0 / (2.0 * KAPPA), scalar2=None,
            op0=ALU.mult,
        )
        pre_all = const.tile([P, T], F32)
        nc.vector.tensor_tensor(
            out=pre_all[:], in0=q_all[:], in1=norm2[:], op=ALU.subtract
        )
        nc.vector.tensor_tensor(
            out=pre_all[:], in0=pre_all[:], in1=bu_all[:, :, 1], op=ALU.subtract
        )
        nc.vector.tensor_tensor(
            out=pre_all[:], in0=pre_all[:], in1=r2[:], op=ALU.mult
        )
        nc.vector.tensor_tensor(
            out=pre_all[:], in0=pre_all[:], in1=bu_all[:, :, 0], op=ALU.add
        )
        con_all = const.tile([P, T], F32)
        nc.scalar.activation(con_all[:], pre_all[:], ACTF.Relu)

        tot_col = mid.tile([P, 1], F32, tag="tot")
        nc.vector.reduce_sum(tot_col[:], con_all[:], axis=mybir.AxisListType.X)
        tot_ps = psmid.tile([1, 1], F32, tag="m")
        nc.tensor.matmul(tot_ps[:], tot_col[:], ones_col[:])
        tot_sb = mid.tile([1, 8], F32, tag="totsb")
        nc.vector.memset(tot_sb[:], 0.0)
        nc.vector.tensor_copy(out=tot_sb[:, 0:1], in_=tot_ps[:])
        nc.sync.dma_start(out=ar2_in.ap()[:], in_=tot_sb[:])
        nc.gpsimd.collective_compute(
            "AllReduce", ALU.add, replica_groups=groups,
            ins=[ar2_in.ap()], outs=[ar2_out.ap()],
        )
        gtot = mid.tile([1, 8], F32, tag="gtot")
        nc.sync.dma_start(out=gtot[:], in_=ar2_out.ap()[:])
        loss_sb = mid.tile([1, 1], F32, tag="loss")
        nc.vector.tensor_tensor(
            out=loss_sb[:], in0=gtot[:, 0:1], in1=inv_np[:], op=ALU.mult
        )
        nc.sync.dma_start(out=loss_out.ap()[:], in_=loss_sb[:])

    nc.compile()
    return nc


_NC = None


def _get_nc():
    global _NC
    if _NC is None:
        _NC = _build()
    return _NC


def build_in_maps(emb: np.ndarray, lab: np.ndarray) -> list[dict]:
    """Shard full inputs across the 8 cores (batch-dim data parallel)."""
    in_maps = []
    for c in range(N_CORES):
        sl = slice(c * B_LOC, (c + 1) * B_LOC)
        lab_c = lab[sl]
        lab_2d = np.ascontiguousarray(lab_c.reshape(T, P).T)  # [P, T]
        wrapped = lab_c.astype(np.int16).reshape(B_LOC // 16, 16).T
        lab16_2d = np.ascontiguousarray(np.tile(wrapped, (P // 16, 1)))
        in_maps.append({"emb": emb[sl], "lab": lab_2d, "lab16": lab16_2d})
    return in_maps


def kernel(embeddings: np.ndarray, labels: np.ndarray) -> np.ndarray:
    emb = np.ascontiguousarray(np.asarray(embeddings, dtype=np.float32))
    lab = np.asarray(labels).astype(np.int32)
    assert emb.shape == (B_FULL, D) and lab.shape == (B_FULL,)

    nc = _get_nc()
    in_maps = build_in_maps(emb, lab)
    res = run_bass_kernel_spmd(nc, in_maps, core_ids=list(range(N_CORES)))
    loss = res.results[0]["loss"]
    return np.asarray(loss, dtype=np.float32).reshape(())


if __name__ == "__main__":
    rng = np.random.default_rng(0)
    e = rng.standard_normal((B_FULL, D), dtype=np.float32)
    l = rng.integers(0, C, size=(B_FULL,)).astype(np.int32)
    print(kernel(embeddings=e, labels=l))


# revision 19
# speedup vs baseline: 1.5705x; 1.0575x over previous
"""Centroid triplet loss on 8 Trainium2 NeuronCores (Bass/Tile).

Data-parallel over the batch: each of the 8 cores gets 8192 of the 65536
samples.  Per-class embedding sums and counts are all-reduced to form global
centroids; each core then computes its local triplet terms and a final
all-reduce produces the scalar loss.

Math restructure (equivalent to the reference):
    term_i = relu(margin + e_hat_i . (cent[nearest[l_i]] - cent[l_i]))
    loss   = sum_i w_{l_i} * term_i / n_present,   w_c = 1/max(count_c, 1)
Since relu(w*x) = w*relu(x) for w > 0, a sample's weighted term is
    relu(b_{l_i} + r_i * (e_i . u_{l_i}))
with u_c = w_c*(cent_near_c - cent_c), b_c = w_c*margin, r_i = 1/||e_i||.
So embeddings stay raw in SBUF; the one-hot used for the class-sum matmul is
scaled by r_i, and pass 2 gathers (u_c, b_c) rows per sample by label and
fuses the dot product via tensor_tensor_reduce.
"""

import sys

for _p in ("/opt/trn_rl_repo",):
    if _p not in sys.path:
        sys.path.insert(0, _p)

from contextlib import ExitStack

import numpy as np

from concourse import bacc, bass, mybir, tile
from concourse.bass_utils import run_bass_kernel_spmd
from concourse.masks import make_identity

F32 = mybir.dt.float32
BF16 = mybir.dt.bfloat16
I32 = mybir.dt.int32
I16 = mybir.dt.int16
ALU = mybir.AluOpType
ACTF = mybir.ActivationFunctionType

N_CORES = 8
B_FULL = 65536
D = 512
C = 256
MARGIN = 0.3
EPS = 1e-12

P = 128                      # SBUF partitions
B_LOC = B_FULL // N_CORES    # 8192 samples per core
T = B_LOC // P               # 64 sample tiles of 128
LOAD_CHUNK = 8               # tiles per embedding-load DMA (2 MiB each)
TBL_B = 640                  # bf16 table row: k*u[0:512], b_hi, b_lo, u2_hi, u2_lo, pad
                             # (1280B, multiple of 256B for dma_gather)
GCHUNK = 1024                # indices per dma_gather call (8 sample tiles)
NEG = -1e30
KAPPA = 256.0                # scale for the difference-of-squares dot trick


def _build():
    nc = bacc.Bacc(
        "TRN2",
        target_bir_lowering=False,
        debug=False,
        enable_asserts=False,
        num_devices=N_CORES,
    )

    emb = nc.dram_tensor("emb", [B_LOC, D], F32, kind="ExternalInput")
    lab = nc.dram_tensor("lab", [P, T], I32, kind="ExternalInput")
    # labels in dma_gather's wrapped-int16 layout: idx i lives at
    # [i % 16, i // 16], replicated into all eight 16-partition groups
    lab16 = nc.dram_tensor("lab16", [P, B_LOC // 16], I16, kind="ExternalInput")
    loss_out = nc.dram_tensor("loss", [1, 1], F32, kind="ExternalOutput")

    # Internal HBM scratch.  AR1 buffer: rows 0:256 per-class sums, row 256
    # carries the per-class counts in its first 256 columns.
    ar1_in = nc.dram_tensor("ar1_in", [C + 1, D], F32)
    ar1_out = nc.dram_tensor("ar1_out", [C + 1, D], F32, addr_space="Shared")
    table = nc.dram_tensor("tbl", [C, TBL_B], BF16)
    ar2_in = nc.dram_tensor("ar2_in", [1, 8], F32)
    ar2_out = nc.dram_tensor("ar2_out", [1, 8], F32, addr_space="Shared")

    groups = [list(range(N_CORES))]

    with tile.TileContext(nc) as tc, ExitStack() as ctx:
        const = ctx.enter_context(tc.tile_pool(name="const", bufs=1))
        big = ctx.enter_context(tc.tile_pool(name="big", bufs=1))
        work = ctx.enter_context(tc.tile_pool(name="work", bufs=3))
        sq = ctx.enter_context(tc.tile_pool(name="sq", bufs=2))
        gat = ctx.enter_context(tc.tile_pool(name="gat", bufs=4))
        mid = ctx.enter_context(tc.tile_pool(name="mid", bufs=1))
        psacc = ctx.enter_context(tc.tile_pool(name="psacc", bufs=1, space="PSUM"))
        psmid = ctx.enter_context(tc.tile_pool(name="psmid", bufs=3, space="PSUM"))

        # ---- constants -------------------------------------------------
        ident = const.tile([P, P], F32)
        make_identity(nc, ident[:])
        iota_row = const.tile([P, C], BF16)
        nc.gpsimd.iota(
            iota_row[:], pattern=[[1, C]], base=0, channel_multiplier=0,
            allow_small_or_imprecise_dtypes=True,
        )
        ones_col = const.tile([P, 1], F32)
        nc.gpsimd.memset(ones_col[:], 1.0)
        ones_col_bf = const.tile([P, 1], BF16)
        nc.gpsimd.memset(ones_col_bf[:], 1.0)
        ones_row = const.tile([1, P], F32)
        nc.gpsimd.memset(ones_row[:], 1.0)

        lab_sb = const.tile([P, T], I32)
        nc.sync.dma_start(out=lab_sb[:], in_=lab.ap())
        lab_f = const.tile([P, T], BF16)
        nc.vector.tensor_copy(out=lab_f[:], in_=lab_sb[:])
        lab16_sb = const.tile([P, B_LOC // 16], I16)
        nc.sync.dma_start(out=lab16_sb[:], in_=lab16.ap())

        # ---- pass 1: load embeddings, norms, class sums/counts ---------
        e_chunks = []
        emb_v = emb.ap().rearrange("(t p) d -> p t d", p=P)
        for ci in range(T // LOAD_CHUNK):
            # bf16 residency: halves SBUF and lets the class-sum matmuls run
            # single-pass bf16 instead of fp32 HI/LO pairs (cast in the DMA,
            # SWDGE-only feature)
            ec = big.tile([P, LOAD_CHUNK, D], BF16, tag=f"e{ci}")
            e_chunks.append(ec)
            sl = slice(ci * LOAD_CHUNK, (ci + 1) * LOAD_CHUNK)
            nc.gpsimd.dma_start(out=ec[:], in_=emb_v[:, sl, :])

        norm2 = const.tile([P, T], F32)
        norm = const.tile([P, T], F32)
        r_all = const.tile([P, T], F32)
        r_bf = const.tile([P, T], BF16)

        sums_ps0 = psacc.tile([P, D], F32, tag="sums0")
        sums_ps1 = psacc.tile([P, D], F32, tag="sums1")
        cnt_ps = psacc.tile([1, C], F32, tag="cnt")

        def e_tile(t):
            return e_chunks[t // LOAD_CHUNK][:, t % LOAD_CHUNK, :]

        for ci in range(T // LOAD_CHUNK):
            csl = slice(ci * LOAD_CHUNK, (ci + 1) * LOAD_CHUNK)
            for j in range(LOAD_CHUNK):
                t = ci * LOAD_CHUNK + j
                sq_t = sq.tile([P, D], F32, tag="sq")
                # tensor_tensor_reduce is broken on this runtime (kills the
                # exec unit) — use ACT Square with free-dim accumulation.
                nc.scalar.activation(
                    sq_t[:], e_tile(t), ACTF.Square,
                    accum_out=norm2[:, t : t + 1],
                )
            # batched per-chunk norm -> r (cheaper than per-tile column ops)
            nc.scalar.activation(norm[:, csl], norm2[:, csl], ACTF.Sqrt)
            nc.vector.reciprocal(r_all[:, csl], norm[:, csl])
            nc.vector.tensor_copy(out=r_bf[:, csl], in_=r_all[:, csl])

            for j in range(LOAD_CHUNK):
                t = ci * LOAD_CHUNK + j
                et = e_tile(t)
                # plain one-hot (tensor_scalar is ~10x slower than broadcast
                # tensor_tensor — use TT against a bf16 iota)
                oht = work.tile([P, C], BF16, tag="oht")
                nc.vector.tensor_tensor(
                    out=oht[:], in0=iota_row[:],
                    in1=lab_f[:, t : t + 1].to_broadcast([P, C]),
                    op=ALU.is_equal,
                )
                # r-scaled one-hot for the normalized class sums; alternate
                # the scaling between ACT and DVE to balance engine load
                osc = work.tile([P, C], BF16, tag="osc")
                if t % 2 == 0:
                    nc.scalar.activation(
                        osc[:], oht[:], ACTF.Copy, scale=r_all[:, t : t + 1]
                    )
                else:
                    nc.vector.tensor_tensor(
                        out=osc[:], in0=oht[:],
                        in1=r_bf[:, t : t + 1].to_broadcast([P, C]),
                        op=ALU.mult,
                    )
                first, last = t == 0, t == T - 1
                nc.tensor.matmul(
                    sums_ps0[:], osc[:, 0:P], et, start=first, stop=last
                )
                nc.tensor.matmul(
                    sums_ps1[:], osc[:, P:C], et, start=first, stop=last
                )
                nc.tensor.matmul(
                    cnt_ps[:], ones_col_bf[:], oht[:], start=first, stop=last
                )

        # ---- all-reduce sums + counts ----------------------------------
        sums_sb = [mid.tile([P, D], F32, tag=f"ssb{h}", name=f"ssb{h}") for h in range(2)]
        nc.vector.tensor_copy(out=sums_sb[0][:], in_=sums_ps0[:])
        nc.vector.tensor_copy(out=sums_sb[1][:], in_=sums_ps1[:])
        cnt_row = mid.tile([1, D], F32, tag="cntrow")
        nc.vector.memset(cnt_row[:], 0.0)
        nc.vector.tensor_copy(out=cnt_row[:, 0:C], in_=cnt_ps[:])

        nc.sync.dma_start(out=ar1_in.ap()[0:P, :], in_=sums_sb[0][:])
        nc.sync.dma_start(out=ar1_in.ap()[P:C, :], in_=sums_sb[1][:])
        nc.sync.dma_start(out=ar1_in.ap()[C : C + 1, :], in_=cnt_row[:])

        nc.gpsimd.collective_compute(
            "AllReduce", ALU.add, replica_groups=groups,
            ins=[ar1_in.ap()], outs=[ar1_out.ap()],
        )

        # global sums overwrite the local-sum tiles (same slots, AR is done)
        gsums = [mid.tile([P, D], F32, tag=f"ssb{h}", name=f"gs{h}") for h in range(2)]
        nc.sync.dma_start(out=gsums[0][:], in_=ar1_out.ap()[0:P, :])
        nc.sync.dma_start(out=gsums[1][:], in_=ar1_out.ap()[P:C, :])
        gcnt_row = mid.tile([1, C], F32, tag="cntrow")
        nc.sync.dma_start(out=gcnt_row[:], in_=ar1_out.ap()[C : C + 1, 0:C])

        # ---- centroids: cent = sums / max(||sums||, eps) ---------------
        cent = []
        for h in range(2):
            s2 = sq.tile([P, D], F32, tag="sq")  # scratch for the squares
            cn2 = mid.tile([P, 1], F32, tag=f"cn{h}")
            nc.scalar.activation(
                s2[:], gsums[h][:], ACTF.Square, accum_out=cn2[:]
            )
            nc.scalar.activation(cn2[:], cn2[:], ACTF.Sqrt)
            nc.vector.tensor_scalar(
                out=cn2[:], in0=cn2[:], scalar1=EPS, scalar2=None, op0=ALU.max
            )
            nc.vector.reciprocal(cn2[:], cn2[:])
            ch = mid.tile([P, D], F32, tag=f"cent{h}")
            nc.vector.tensor_scalar(
                out=ch[:], in0=gsums[h][:], scalar1=cn2[:], scalar2=None,
                op0=ALU.mult,
            )
            cent.append(ch)

        # ---- presence masks, counts columns, w -------------------------
        negmask_r = mid.tile([1, C], F32, tag="negm")
        nc.vector.tensor_scalar(
            out=negmask_r[:], in0=gcnt_row[:], scalar1=0.5, scalar2=float(NEG),
            op0=ALU.is_lt, op1=ALU.mult,
        )
        present_r = mid.tile([1, C], F32, tag="pres")
        nc.vector.tensor_scalar(
            out=present_r[:], in0=gcnt_row[:], scalar1=0.5, scalar2=None,
            op0=ALU.is_ge,
        )
        npres = mid.tile([1, 1], F32, tag="npres")
        nc.vector.reduce_sum(npres[:], present_r[:], axis=mybir.AxisListType.X)
        nc.vector.tensor_scalar(
            out=npres[:], in0=npres[:], scalar1=1.0, scalar2=None, op0=ALU.max
        )
        inv_np = mid.tile([1, 1], F32, tag="invnp")
        nc.vector.reciprocal(inv_np[:], npres[:])

        wcol = []
        for h in range(2):
            ccol_ps = psmid.tile([P, 1], F32, tag="m")
            nc.tensor.matmul(
                ccol_ps[:], gcnt_row[:, h * P : (h + 1) * P], ones_row[:, 0:1]
            )
            wc = mid.tile([P, 1], F32, tag=f"w{h}")
            nc.vector.tensor_scalar(
                out=wc[:], in0=ccol_ps[:], scalar1=1.0, scalar2=None, op0=ALU.max
            )
            nc.vector.reciprocal(wc[:], wc[:])
            wcol.append(wc)

        # ---- centroid similarity G = cent @ cent.T ---------------------
        centT = [mid.tile([P, C], F32, tag=f"ct{k}", name=f"ct{k}") for k in range(4)]
        for h in range(2):
            for k in range(4):
                tp = psmid.tile([P, P], F32, tag="m")
                nc.tensor.transpose(
                    tp[:], cent[h][:, k * P : (k + 1) * P], ident[:]
                )
                nc.vector.tensor_copy(
                    out=centT[k][:, h * P : (h + 1) * P], in_=tp[:]
                )

        g_sb = []
        for h in range(2):
            gp = psmid.tile([P, C], F32, tag="m")
            for k in range(4):
                nc.tensor.matmul(
                    gp[:], centT[k][:, h * P : (h + 1) * P], centT[k][:],
                    start=(k == 0), stop=(k == 3),
                )
            gs = mid.tile([P, C], F32, tag=f"g{h}")
            nc.vector.tensor_copy(out=gs[:], in_=gp[:])
            # mask the diagonal (self-similarity): keep where col - row != 0
            nc.gpsimd.affine_select(
                out=gs[:], in_=gs[:], compare_op=ALU.not_equal, fill=NEG,
                base=-h * P, pattern=[[1, C]], channel_multiplier=-1,
            )
            g_sb.append(gs)

        # add -1e30 to columns of empty classes (broadcast the row via PE)
        maskp = psmid.tile([P, C], F32, tag="m")
        nc.tensor.matmul(maskp[:], ones_row[:], negmask_r[:])
        for h in range(2):
            nc.vector.tensor_tensor(
                out=g_sb[h][:], in0=g_sb[h][:], in1=maskp[:], op=ALU.add
            )

        # ---- nearest-centroid one-hot (argmax by equality) -------------
        nst = [mid.tile([P, C], F32, tag=f"nst{k}", name=f"nst{k}") for k in range(2)]
        for h in range(2):
            mx = mid.tile([P, 1], F32, tag=f"mx{h}")
            nc.vector.reduce_max(mx[:], g_sb[h][:], axis=mybir.AxisListType.X)
            ns = mid.tile([P, C], F32, tag=f"ns{h}")
            nc.vector.tensor_scalar(
                out=ns[:], in0=g_sb[h][:], scalar1=mx[:], scalar2=None,
                op0=ALU.is_equal,
            )
            for k in range(2):
                tp = psmid.tile([P, P], F32, tag="m")
                nc.tensor.transpose(tp[:], ns[:, k * P : (k + 1) * P], ident[:])
                nc.vector.tensor_copy(
                    out=nst[k][:, h * P : (h + 1) * P], in_=tp[:]
                )

        # ---- u = w*(cent_near - cent), b = w*margin; write table -------
        for h in range(2):
            cnear = psmid.tile([P, D], F32, tag="m")
            for k in range(2):
                nc.tensor.matmul(
                    cnear[:], nst[k][:, h * P : (h + 1) * P], cent[k][:],
                    start=(k == 0), stop=(k == 1),
                )
            # k*u in fp32, then round to the bf16 row; b and k^2|u|^2 are
            # stored as bf16 hi+lo pairs to keep fp32-level precision
            uf = mid.tile([P, D], F32, tag="uf")
            nc.vector.tensor_tensor(
                out=uf[:], in0=cnear[:], in1=cent[h][:], op=ALU.subtract
            )
            nc.vector.tensor_scalar(
                out=uf[:], in0=uf[:], scalar1=wcol[h][:],
                scalar2=KAPPA, op0=ALU.mult, op1=ALU.mult,
            )
            tbl_sb = mid.tile([P, TBL_B], BF16, tag=f"tb{h}")
            nc.vector.tensor_copy(out=tbl_sb[:, 0:D], in_=uf[:])
            bcol = mid.tile([P, 1], F32, tag=f"bc{h}")
            nc.vector.tensor_scalar(
                out=bcol[:], in0=wcol[h][:], scalar1=MARGIN, scalar2=None,
                op0=ALU.mult,
            )
            u2col = mid.tile([P, 1], F32, tag=f"u2{h}")
            squ = sq.tile([P, D], F32, tag="sq")
            nc.scalar.activation(
                squ[:], tbl_sb[:, 0:D], ACTF.Square, accum_out=u2col[:]
            )
            lo = mid.tile([P, 1], F32, tag=f"lo{h}")
            nc.vector.tensor_copy(out=tbl_sb[:, D : D + 1], in_=bcol[:])
            nc.vector.tensor_copy(out=lo[:], in_=tbl_sb[:, D : D + 1])
            nc.vector.tensor_tensor(out=lo[:], in0=bcol[:], in1=lo[:],
                                    op=ALU.subtract)
            nc.vector.tensor_copy(out=tbl_sb[:, D + 1 : D + 2], in_=lo[:])
            nc.vector.tensor_copy(out=tbl_sb[:, D + 2 : D + 3], in_=u2col[:])
            nc.vector.tensor_copy(out=lo[:], in_=tbl_sb[:, D + 2 : D + 3])
            nc.vector.tensor_tensor(out=lo[:], in0=u2col[:], in1=lo[:],
                                    op=ALU.subtract)
            nc.vector.tensor_copy(out=tbl_sb[:, D + 3 : D + 4], in_=lo[:])
            nc.vector.memset(tbl_sb[:, D + 4 : TBL_B], 0.0)
            nc.sync.dma_start(out=table.ap()[h * P : (h + 1) * P, :], in_=tbl_sb[:])

        # ---- pass 2: gather (k*u, b, k^2|u|^2) by label; dot via the ----
        # difference of squares:  e.u = (|e + k*u|^2 - |e|^2 - k^2|u|^2)/2k.
        # (tensor_tensor_reduce is broken on HW; multi-index indirect
        # gathers too — one [P,1]-offset gather per 128-sample tile.)
        q_all = const.tile([P, T], F32)
        bu_all = const.tile([P, T, 4], F32)
        tiles_per_g = GCHUNK // P
        for gc in range(T // tiles_per_g):
            g_t = gat.tile([P, tiles_per_g, TBL_B], BF16, tag="g", name=f"g{gc}")
            nc.gpsimd.dma_gather(
                out_ap=g_t[:], in_ap=table.ap(),
                idxs_ap=lab16_sb[:, gc * (GCHUNK // 16) : (gc + 1) * (GCHUNK // 16)],
                num_idxs=GCHUNK, num_idxs_reg=GCHUNK, elem_size=TBL_B,
            )
            nc.vector.tensor_copy(
                out=bu_all[:, gc * tiles_per_g : (gc + 1) * tiles_per_g, :],
                in_=g_t[:, :, D : D + 4],
            )
            for j in range(tiles_per_g):
                t = gc * tiles_per_g + j
                s_t = sq.tile([P, D], F32, tag="pr")
                nc.vector.tensor_tensor(
                    out=s_t[:], in0=e_tile(t), in1=g_t[:, j, 0:D], op=ALU.add
                )
                sq2 = sq.tile([P, D], F32, tag="sq")
                nc.scalar.activation(
                    sq2[:], s_t[:], ACTF.Square, accum_out=q_all[:, t : t + 1]
                )


        # pre = (q - |e|^2 - k^2|u|^2) * (r / 2k) + b ;  term = relu(pre)
        r2 = const.tile([P, T], F32)
        nc.vector.tensor_scalar(
            out=r2[:], in0=r_all[:], scalar1=1.0 / (2.0 * KAPPA), scalar2=None,
            op0=ALU.mult,
        )
        pre_all = const.tile([P, T], F32)
        nc.vector.tensor_tensor(
            out=pre_all[:], in0=q_all[:], in1=norm2[:], op=ALU.subtract
        )
        nc.vector.tensor_tensor(
            out=pre_all[:], in0=pre_all[:], in1=bu_all[:, :, 2], op=ALU.subtract
        )
        nc.vector.tensor_tensor(
            out=pre_all[:], in0=pre_all[:], in1=bu_all[:, :, 3], op=ALU.subtract
        )
        nc.vector.tensor_tensor(
            out=pre_all[:], in0=pre_all[:], in1=r2[:], op=ALU.mult
        )
        nc.vector.tensor_tensor(
            out=pre_all[:], in0=pre_all[:], in1=bu_all[:, :, 0], op=ALU.add
        )
        nc.vector.tensor_tensor(
            out=pre_all[:], in0=pre_all[:], in1=bu_all[:, :, 1], op=ALU.add
        )
        con_all = const.tile([P, T], F32)
        nc.scalar.activation(con_all[:], pre_all[:], ACTF.Relu)

        tot_col = mid.tile([P, 1], F32, tag="tot")
        nc.vector.reduce_sum(tot_col[:], con_all[:], axis=mybir.AxisListType.X)
        tot_ps = psmid.tile([1, 1], F32, tag="m")
        nc.tensor.matmul(tot_ps[:], tot_col[:], ones_col[:])
        tot_sb = mid.tile([1, 8], F32, tag="totsb")
        nc.vector.memset(tot_sb[:], 0.0)
        nc.vector.tensor_copy(out=tot_sb[:, 0:1], in_=tot_ps[:])
        nc.sync.dma_start(out=ar2_in.ap()[:], in_=tot_sb[:])
        nc.gpsimd.collective_compute(
            "AllReduce", ALU.add, replica_groups=groups,
            ins=[ar2_in.ap()], outs=[ar2_out.ap()],
        )
        gtot = mid.tile([1, 8], F32, tag="gtot")
        nc.sync.dma_start(out=gtot[:], in_=ar2_out.ap()[:])
        loss_sb = mid.tile([1, 1], F32, tag="loss")
        nc.vector.tensor_tensor(
            out=loss_sb[:], in0=gtot[:, 0:1], in1=inv_np[:], op=ALU.mult
        )
        nc.sync.dma_start(out=loss_out.ap()[:], in_=loss_sb[:])

    nc.compile()
    return nc


_NC = None


def _get_nc():
    global _NC
    if _NC is None:
        _NC = _build()
    return _NC


def build_in_maps(emb: np.ndarray, lab: np.ndarray) -> list[dict]:
    """Shard full inputs across the 8 cores (batch-dim data parallel)."""
    in_maps = []
    for c in range(N_CORES):
        sl = slice(c * B_LOC, (c + 1) * B_LOC)
        lab_c = lab[sl]
        lab_2d = np.ascontiguousarray(lab_c.reshape(T, P).T)  # [P, T]
        wrapped = lab_c.astype(np.int16).reshape(B_LOC // 16, 16).T
        lab16_2d = np.ascontiguousarray(np.tile(wrapped, (P // 16, 1)))
        in_maps.append({"emb": emb[sl], "lab": lab_2d, "lab16": lab16_2d})
    return in_maps


def kernel(embeddings: np.ndarray, labels: np.ndarray) -> np.ndarray:
    emb = np.ascontiguousarray(np.asarray(embeddings, dtype=np.float32))
    lab = np.asarray(labels).astype(np.int32)
    assert emb.shape == (B_FULL, D) and lab.shape == (B_FULL,)

    nc = _get_nc()
    in_maps = build_in_maps(emb, lab)
    res = run_bass_kernel_spmd(nc, in_maps, core_ids=list(range(N_CORES)))
    loss = res.results[0]["loss"]
    return np.asarray(loss, dtype=np.float32).reshape(())


if __name__ == "__main__":
    rng = np.random.default_rng(0)
    e = rng.standard_normal((B_FULL, D), dtype=np.float32)
    l = rng.integers(0, C, size=(B_FULL,)).astype(np.int32)
    print(kernel(embeddings=e, labels=l))
